# revision 11
# baseline (speedup 1.0000x reference)
"""Trainium2 Bass kernel for nn_DKT_89034672046889 (DKT-style recurrent net).

Strategy: data-parallel over batch across 8 NeuronCores (128 rows each).
On-device layout is feature-major ([feature, t*128+b]); host does
weight-only precompute (sigmoid tables, answer-embedding folds, gate-folded
recurrent weights) and input layout transforms (transpose/cast/shard).
Both scans are expressed with sigmoid-only activations:
  scan1 state v_t = sigma(2*a_t)        (u_t = tanh(a_t) = 2 v_t - 1)
  scan2 state p_t = (1+h_t)/2 in [0,1]  (p' = (1-z) p + z w)
so the ACT engine never switches function tables.
Table gathers use split-range int16 dma_gather(transpose=True) with
zero-row dummy slots, merged by a single add -- results land feature-major.
"""

import sys

for _p in ("/root/.axon_site/_ro/trn_rl_repo", "/opt/trn_rl_repo"):
    if _p not in sys.path:
        sys.path.append(_p)

import numpy as np
import ml_dtypes

import concourse.bacc as bacc
import concourse.mybir as mybir
import concourse.tile as tile
from concourse.bass import IndirectOffsetOnAxis
from concourse.bass_utils import run_bass_kernel_spmd

BF = mybir.dt.bfloat16
F32 = mybir.dt.float32
I32 = mybir.dt.int32
I16 = mybir.dt.int16

B, T, H, EMB = 1024, 39, 123, 256
NCORES = 8
BL = B // NCORES            # 128 batch rows per core
NT = T * BL                 # 4992 columns, t-major (n = t*128 + b)
GSZ = 512
SPLIT = 32767               # int16 index split for dma_gather

_bf16 = ml_dtypes.bfloat16

# bf16 weight-bundle column layout: name -> (col0, ncols, nparts)
_WB = {}
_c = 0
for _n, _w, _p in (("Lzz", H, 124), ("Lxp", H, 125), ("Lz", H, 125),
                   ("Lh2", H, 125), ("Az", H, H), ("Ah", H, H),
                   ("o1a", EMB, H), ("o1c", EMB, H), ("p2T", 2 * H, 128),
                   ("p3g", 1, 124), ("e127", H, 128), ("identb", 128, 128)):
    _WB[_n] = (_c, _w, _p)
    _c += _w
WB_COLS = _c
# f32 bundle
_FB = {}
_c = 0
for _n, _w, _p in (("GA", T, H), ("GB", T, H), ("o1b", 2, 128),
                   ("p2b", 1, H), ("dtv", 1, H), ("ident", 128, 128)):
    _FB[_n] = (_c, _w, _p)
    _c += _w
FB_COLS = _c


def _sigmoid(x):
    return 1.0 / (1.0 + np.exp(-x))


def _groups():
    out = []
    c = 0
    while c < NT:
        s = min(GSZ, NT - c)
        out.append((c, s))
        c += s
    return out


def build_nc(dbg=False):
    nc = bacc.Bacc(None, target_bir_lowering=False, debug=False)

    dt = nc.dram_tensor
    qmT_d = dt("qmT", [124, NT], BF, kind="ExternalInput")       # row 123 = ones
    qmnT_d = dt("qmnT", [123, NT], BF, kind="ExternalInput")
    rr2_d = dt("rr2", [2, NT], BF, kind="ExternalInput")         # [r_row; ones]
    onesr_d = dt("onesr", [1, (T + 1) * BL], BF, kind="ExternalInput")
    sid_d = dt("sid", [BL, 1], I32, kind="ExternalInput")
    ixka_d = dt("ixka", [128, NT // 16], I16, kind="ExternalInput")
    ixkb_d = dt("ixkb", [128, NT // 16], I16, kind="ExternalInput")
    ixea_d = dt("ixea", [128, NT // 16], I16, kind="ExternalInput")
    ixeb_d = dt("ixeb", [128, NT // 16], I16, kind="ExternalInput")
    stab_d = dt("stab", [100001, H], F32, kind="ExternalInput")
    kdx_d = dt("kdx", [50003, 128], BF, kind="ExternalInput")
    e3x_d = dt("e3x", [50003, EMB], BF, kind="ExternalInput")
    wb_d = dt("wb", [128, WB_COLS], BF, kind="ExternalInput")
    fb_d = dt("fb", [128, FB_COLS], F32, kind="ExternalInput")
    A1_d = dt("A1", [124, T * H], BF, kind="ExternalInput")

    out_d = dt("out", [BL, T], F32, kind="ExternalOutput")
    dbg_d = {}
    if dbg:
        for nm, shp in (("dRHS1", [125, NT]), ("dV", [124, (T + 1) * BL]),
                        ("dRHS2", [125, NT]), ("dP", [H, (T + 1) * BL]),
                        ("dO1", [128, 2 * NT]), ("dO2", [124, NT]),
                        ("dDKT", [128, NT]), ("dqmDT", [H, NT]),
                        ("dDKQ", [H, NT]), ("dE3T", [128, 2 * NT])):
            dbg_d[nm] = dt(nm, shp, BF, kind="ExternalOutput")
        dbg_d["dSPT"] = dt("dSPT", [H, BL], F32, kind="ExternalOutput")

    groups = _groups()

    with tile.TileContext(nc) as tc:
        with (
            tc.tile_pool(name="per", bufs=1) as per,
            tc.tile_pool(name="tmp", bufs=4) as tmp,
            tc.tile_pool(name="zw", bufs=3) as zwp,
            tc.tile_pool(name="psA", bufs=2, space="PSUM") as psA,
            tc.tile_pool(name="psPT", bufs=2, space="PSUM") as psPT,
            tc.tile_pool(name="psT", bufs=1, space="PSUM") as psT,
            tc.tile_pool(name="psO1", bufs=1, space="PSUM") as psO1,
            tc.tile_pool(name="psO2", bufs=1, space="PSUM") as psO2,
            tc.tile_pool(name="psO3", bufs=1, space="PSUM") as psO3,
        ):
            sync, gp, ve, se, te = nc.sync, nc.gpsimd, nc.vector, nc.scalar, nc.tensor
            SIG = mybir.ActivationFunctionType.Sigmoid
            MUL = mybir.AluOpType.mult
            ADD = mybir.AluOpType.add
            SUB = mybir.AluOpType.subtract

            # ---- persistent SBUF tiles ----
            qmT = per.tile([124, NT], BF)
            qmnT = per.tile([123, NT], BF)
            RHS1 = per.tile([125, NT], BF)
            RHS2 = per.tile([125, NT], BF)
            V = per.tile([124, (T + 1) * BL], BF)
            P = per.tile([H, (T + 1) * BL], BF)
            DKT = per.tile([128, NT], BF)
            DKB = per.tile([128, NT], BF)
            E3T = per.tile([128, 2 * NT], BF)
            E3B = per.tile([128, 2 * NT], BF)
            qmDT = per.tile([H, NT], BF)
            DKQ = per.tile([H, NT], BF)
            O1 = per.tile([128, 2 * NT], BF)
            O2 = per.tile([124, NT], BF)
            sidt = per.tile([BL, 1], I32)
            ixka = per.tile([128, NT // 16], I16)
            ixkb = per.tile([128, NT // 16], I16)
            ixea = per.tile([128, NT // 16], I16)
            ixeb = per.tile([128, NT // 16], I16)
            WB = per.tile([128, WB_COLS], BF)
            FB = per.tile([128, FB_COLS], F32)
            A1 = per.tile([124, T * H], BF)
            SPT = per.tile([H, BL], F32)
            SPD = per.tile([H, BL], BF)
            SPT4 = per.tile([H, GSZ], F32)
            OUTt = per.tile([BL, T], F32)

            def wb(nm, parts=None):
                c0, w, p = _WB[nm]
                return WB[0:(parts or p), c0:c0 + w]

            def fb(nm, parts=None):
                c0, w, p = _FB[nm]
                return FB[0:(parts or p), c0:c0 + w]

            # ---- small init ----
            ve.memset(V[0:H, 0:BL], 0.0)
            ve.memset(P[:, 0:BL], 0.5)

            # ---- loads ----
            sync.dma_start(out=sidt[:], in_=sid_d[:])
            sync.dma_start(out=ixka[:], in_=ixka_d[:])
            sync.dma_start(out=ixkb[:], in_=ixkb_d[:])
            sync.dma_start(out=ixea[:], in_=ixea_d[:])
            sync.dma_start(out=ixeb[:], in_=ixeb_d[:])
            sync.dma_start(out=WB[:], in_=wb_d[:])
            sync.dma_start(out=FB[:], in_=fb_d[:])
            sync.dma_start(out=A1[:], in_=A1_d[:])
            sync.dma_start(out=qmT[:], in_=qmT_d[:])
            sync.dma_start(out=qmnT[:], in_=qmnT_d[:])
            sync.dma_start(out=RHS1[123:125, :], in_=rr2_d[:])
            sync.dma_start(out=RHS2[123:125, :], in_=rr2_d[:])
            sync.dma_start(out=V[123:124, :], in_=onesr_d[:])
            sync.dma_start(out=O2[123:124, :], in_=onesr_d[:, 0:NT])

            # ---- gathers (feature-major via transposed dma_gather) ----
            gp.dma_gather(
                out_ap=DKT[:].rearrange("p (c n) -> p c n", c=1),
                in_ap=kdx_d[0:SPLIT + 1, :], idxs_ap=ixka[:],
                num_idxs=NT, num_idxs_reg=NT, elem_size=128,
                transpose=True, single_packet=False)
            gp.dma_gather(
                out_ap=DKB[:].rearrange("p (c n) -> p c n", c=1),
                in_ap=kdx_d[SPLIT + 1:, :], idxs_ap=ixkb[:],
                num_idxs=NT, num_idxs_reg=NT, elem_size=128,
                transpose=True, single_packet=False)
            ve.tensor_tensor(out=DKT[:], in0=DKT[:], in1=DKB[:], op=ADD)
            gp.dma_gather(
                out_ap=E3T[:].rearrange("p (c n) -> p c n", c=2),
                in_ap=e3x_d[0:SPLIT + 1, :], idxs_ap=ixea[:],
                num_idxs=NT, num_idxs_reg=NT, elem_size=EMB,
                transpose=True, single_packet=False)
            gp.dma_gather(
                out_ap=E3B[:].rearrange("p (c n) -> p c n", c=2),
                in_ap=e3x_d[SPLIT + 1:, :], idxs_ap=ixeb[:],
                num_idxs=NT, num_idxs_reg=NT, elem_size=EMB,
                transpose=True, single_packet=False)
            ve.tensor_tensor(out=E3T[:], in0=E3T[:], in1=E3B[:], op=ADD)

            # ---- student profile (indirect gather + PE transpose) ----
            sg = per.tile([BL, H], F32)
            gp.indirect_dma_start(
                out=sg[:], out_offset=None, in_=stab_d[:],
                in_offset=IndirectOffsetOnAxis(ap=sidt[:, 0:1], axis=0))
            pT = psT.tile([124, 128], F32, space="PSUM", tag="tp")
            te.transpose(out=pT[0:H, :], in_=sg[:], identity=fb("ident"))
            ve.tensor_copy(out=SPT[:], in_=pT[0:H, :])
            ve.tensor_scalar(out=SPD[:], in0=SPT[:], scalar1=fb("dtv"),
                             scalar2=None, op0=MUL)
            for i in range(4):
                ve.tensor_copy(out=SPT4[:, i * BL:(i + 1) * BL], in_=SPT[:])

            # ---- concept_ab + x1a -> RHS1 rows 0:123 ----
            for (c0, csz) in groups:
                pa = psA.tile([H, GSZ], F32, space="PSUM", tag="a")
                te.matmul(out=pa[:, 0:csz], lhsT=wb("Lzz"), rhs=qmT[:, c0:c0 + csz],
                          start=True, stop=True)
                ve.tensor_tensor(out=RHS1[0:H, c0:c0 + csz], in0=pa[:, 0:csz],
                                 in1=SPT4[:, 0:csz], op=MUL)

            # ---- qmDT / DKQ per group + scan1 + middle (pipelined emission) ----
            ps1 = [None]
            for t in range(T + 4):
                if t < T:
                    if t % 4 == 0:
                        g = t // 4
                        c0, csz = groups[g]
                        pd = psA.tile([H, GSZ], F32, space="PSUM", tag="a",
                                      name="pdg")
                        te.matmul(out=pd[:, 0:csz], lhsT=wb("e127"),
                                  rhs=DKT[:, c0:c0 + csz], start=True, stop=True)
                        ve.tensor_tensor(out=qmDT[:, c0:c0 + csz], in0=pd[:, 0:csz],
                                         in1=qmT[0:H, c0:c0 + csz], op=MUL)
                        ve.tensor_tensor(out=DKQ[:, c0:c0 + csz],
                                         in0=DKT[0:H, c0:c0 + csz],
                                         in1=qmT[0:H, c0:c0 + csz], op=MUL)
                    # scan1 tick t
                    gi, off = divmod(t, 4)
                    if off == 0:
                        c0 = gi * GSZ
                        csz = min(GSZ, NT - c0)
                        ps1[0] = psA.tile([H, GSZ], F32, space="PSUM", tag="a",
                                          name="ps1g")
                        te.matmul(out=ps1[0][:, 0:csz], lhsT=wb("Lxp"),
                                  rhs=RHS1[:, c0:c0 + csz], start=True, stop=True)
                    sl = slice(off * BL, (off + 1) * BL)
                    te.matmul(out=ps1[0][:, sl], lhsT=A1[:, t * H:(t + 1) * H],
                              rhs=V[:, t * BL:(t + 1) * BL], start=False, stop=True)
                    se.activation(out=V[0:H, (t + 1) * BL:(t + 2) * BL],
                                  in_=ps1[0][:, sl], func=SIG)
                # middle, lagged 4 ticks
                tm = t - 4
                if 0 <= tm < T:
                    m1 = tmp.tile([H, BL], BF, tag="m1")
                    ve.tensor_scalar(out=m1[:],
                                     in0=V[0:H, (tm + 1) * BL:(tm + 2) * BL],
                                     scalar1=fb("GA")[:, tm:tm + 1],
                                     scalar2=fb("GB")[:, tm:tm + 1],
                                     op0=MUL, op1=ADD)
                    m2 = tmp.tile([H, BL], BF, tag="m2")
                    ve.tensor_tensor(out=m2[:], in0=m1[:], in1=SPD[:], op=ADD)
                    m3 = tmp.tile([H, BL], BF, tag="m3")
                    ve.tensor_tensor(out=m3[:], in0=m2[:],
                                     in1=qmDT[:, tm * BL:(tm + 1) * BL], op=MUL)
                    ve.tensor_tensor(out=RHS2[0:H, tm * BL:(tm + 1) * BL], in0=m3[:],
                                     in1=DKQ[:, tm * BL:(tm + 1) * BL], op=SUB)

            # ---- scan2 + o1/o2 pipelined ----
            for t in range(T + 8):
                if t < T:
                    pt = psPT.tile([H, 256], F32, space="PSUM", tag="pt")
                    te.matmul(out=pt[:, 0:128], lhsT=wb("Lz"),
                              rhs=RHS2[:, t * BL:(t + 1) * BL], start=True, stop=True)
                    te.matmul(out=pt[:, 128:256], lhsT=wb("Lh2"),
                              rhs=RHS2[:, t * BL:(t + 1) * BL], start=False, stop=True)
                    te.matmul(out=pt[:, 0:128], lhsT=wb("Az"),
                              rhs=P[:, t * BL:(t + 1) * BL], start=False, stop=True)
                    te.matmul(out=pt[:, 128:256], lhsT=wb("Ah"),
                              rhs=P[:, t * BL:(t + 1) * BL], start=False, stop=True)
                    zw = zwp.tile([H, 256], BF, tag="zw")
                    se.activation(out=zw[:], in_=pt[:], func=SIG)
                    d1 = tmp.tile([H, BL], BF, tag="d1")
                    ve.tensor_tensor(out=d1[:], in0=zw[:, 128:256],
                                     in1=P[:, t * BL:(t + 1) * BL], op=SUB)
                    d2 = tmp.tile([H, BL], BF, tag="d2")
                    ve.tensor_tensor(out=d2[:], in0=zw[:, 0:128], in1=d1[:], op=MUL)
                    ve.tensor_tensor(out=P[:, (t + 1) * BL:(t + 2) * BL],
                                     in0=P[:, t * BL:(t + 1) * BL], in1=d2[:], op=ADD)
                if t >= 4 and (t - 4) % 4 == 0:
                    g = (t - 4) // 4
                    if g < len(groups):
                        c0, csz = groups[g]
                        for m in range(2):
                            po = psO1.tile([128, GSZ], F32, space="PSUM", tag="o1")
                            te.matmul(out=po[:, 0:csz], lhsT=wb("identb"),
                                      rhs=E3T[:, m * NT + c0:m * NT + c0 + csz],
                                      start=True, stop=True)
                            te.matmul(out=po[:, 0:csz],
                                      lhsT=wb("o1a")[:, m * 128:(m + 1) * 128],
                                      rhs=P[:, BL + c0:BL + c0 + csz],
                                      start=False, stop=True)
                            te.matmul(out=po[:, 0:csz],
                                      lhsT=wb("o1c")[:, m * 128:(m + 1) * 128],
                                      rhs=qmnT[:, c0:c0 + csz], start=False, stop=True)
                            se.activation(out=O1[:, m * NT + c0:m * NT + c0 + csz],
                                          in_=po[:, 0:csz], func=SIG,
                                          bias=fb("o1b")[:, m:m + 1])
                if t >= 8 and (t - 8) % 4 == 0:
                    g = (t - 8) // 4
                    if g < len(groups):
                        c0, csz = groups[g]
                        p2 = psO2.tile([H, GSZ], F32, space="PSUM", tag="o2")
                        te.matmul(out=p2[:, 0:csz], lhsT=wb("p2T")[:, 0:H],
                                  rhs=O1[:, c0:c0 + csz], start=True, stop=True)
                        te.matmul(out=p2[:, 0:csz], lhsT=wb("p2T")[:, H:2 * H],
                                  rhs=O1[:, NT + c0:NT + c0 + csz],
                                  start=False, stop=True)
                        se.activation(out=O2[0:H, c0:c0 + csz], in_=p2[:, 0:csz],
                                      func=SIG, bias=fb("p2b"))

            # ---- o3 ----
            p3 = psO3.tile([128, T], F32, space="PSUM", tag="o3")
            for t in range(T):
                te.matmul(out=p3[:, t:t + 1], lhsT=O2[:, t * BL:(t + 1) * BL],
                          rhs=wb("p3g"), start=(t == 0), stop=True)
            se.activation(out=OUTt[:], in_=p3[:], func=SIG)
            sync.dma_start(out=out_d[:], in_=OUTt[:])
            if dbg:
                for nm, src_tile in (("dRHS1", RHS1), ("dV", V), ("dRHS2", RHS2),
                                     ("dP", P), ("dO1", O1), ("dO2", O2),
                                     ("dDKT", DKT), ("dqmDT", qmDT),
                                     ("dDKQ", DKQ), ("dE3T", E3T)):
                    sync.dma_start(out=dbg_d[nm][:], in_=src_tile[:])
                sync.dma_start(out=dbg_d["dSPT"][:], in_=SPT[:])

    nc.finalize()
    return nc


def _wrap_idx(a):
    """flat [NT] int16 -> [128, NT//16] wrapped+replicated layout."""
    return np.ascontiguousarray(np.tile(a.reshape(NT // 16, 16).T, (8, 1)))


def _split_idx(idx):
    """flat indices -> (idxA, idxB) int16 with zero-row dummies."""
    a = np.where(idx < SPLIT, idx + 1, 0).astype(np.int16)
    b = np.where(idx >= SPLIT, idx - SPLIT + 1, 0).astype(np.int16)
    return _wrap_idx(a), _wrap_idx(b)


def _xtab(tbl):
    """[R, W] table -> [R+2, W] with zero rows at 0 and SPLIT+1."""
    R, W = tbl.shape
    out = np.zeros((R + 2, W), tbl.dtype)
    out[1:SPLIT + 1] = tbl[0:SPLIT]
    out[SPLIT + 2:] = tbl[SPLIT:]
    return out


def host_prep(inputs):
    """Weight-only precompute + input layout transforms. Returns in_maps."""
    f = lambda k: np.asarray(inputs[k], np.float32)
    ii = lambda k: np.asarray(inputs[k]).astype(np.int64)

    d_t = float(f("d_t")[0])
    d_e = float(f("d_e")[0])
    W_ih, b_ih = f("W_ih"), f("b_ih")
    W_hh, b_hh = f("W_hh"), f("b_hh")
    W_z, b_z = f("W_z"), f("b_z")
    W_h, b_h = f("W_h"), f("b_h")
    answer_W = f("answer_W")
    zz_W, zz_b = f("zz_W"), f("zz_b")
    p1_W, p1_b = f("p1_W"), f("p1_b")
    p2_W, p2_b = f("p2_W"), f("p2_b")
    p3_W, p3_b = f("p3_W"), f("p3_b")
    W_tg, b_tg = f("W_tg"), f("b_tg")

    tvec = np.arange(T, dtype=np.float32)[:, None]
    G = _sigmoid(tvec * W_tg[:, 0][None, :] + b_tg)          # [T,123]

    stab = _sigmoid(f("student_W")).astype(np.float32)
    D_tab = _sigmoid(f("e_disc_W")[:, 0]) * d_e
    kdt = np.zeros((50001, 128), np.float32)
    kdt[:, 0:H] = _sigmoid(f("k_diff_W")) * D_tab[:, None]
    kdt[:, H] = D_tab
    e3t = (f("emb_problem") @ p1_W[:, 123:379].T).astype(np.float32)

    def fold(Wm, bias):
        ap = answer_W @ Wm[:, 123:379].T
        return ap[0] + bias, ap[1] - ap[0]
    c0_ih, dl_ih = fold(W_ih, b_ih)
    c0_z, dl_z = fold(W_z, b_z)
    c0_h, dl_h = fold(W_h, b_h)
    Wz_h = W_z[:, 379:502]
    Wh_h = W_h[:, 379:502]

    A1_flat = np.zeros((124, T * H), np.float32)
    A1_flat[123, 0:H] = 2 * b_hh
    for t in range(1, T):
        g = G[t - 1]
        A1_flat[:123, t * H:(t + 1) * H] = 4.0 * g[:, None] * W_hh.T
        A1_flat[123, t * H:(t + 1) * H] = 2 * b_hh - 2.0 * (W_hh @ g)

    bf = lambda x: np.ascontiguousarray(x, np.float32).astype(_bf16)

    wbund = np.zeros((128, WB_COLS), np.float32)

    def put_wb(nm, mat):
        c0, w, p = _WB[nm]
        assert mat.shape == (p, w), (nm, mat.shape)
        wbund[0:p, c0:c0 + w] = mat

    put_wb("Lzz", np.concatenate([zz_W.T, zz_b[None]], 0))
    put_wb("Lxp", np.concatenate(
        [2 * W_ih[:, :123].T, 2 * dl_ih[None], 2 * c0_ih[None]], 0))
    put_wb("Lz", np.concatenate(
        [W_z[:, :123].T, dl_z[None], (c0_z - Wz_h.sum(1))[None]], 0))
    put_wb("Lh2", np.concatenate(
        [2 * W_h[:, :123].T, 2 * dl_h[None], (2 * c0_h - 2 * Wh_h.sum(1))[None]], 0))
    put_wb("Az", 2.0 * Wz_h.T)
    put_wb("Ah", 4.0 * Wh_h.T)
    put_wb("o1a", 2.0 * p1_W[:, :123].T)
    put_wb("o1c", p1_W[:, 379:502].T)
    p2Tm = np.zeros((128, 2 * H), np.float32)
    p2Tm[:, 0:H] = p2_W.T[0:128]
    p2Tm[:, H:2 * H] = p2_W.T[128:256]
    put_wb("p2T", p2Tm)
    put_wb("p3g", np.concatenate([p3_W[0], p3_b]).reshape(124, 1))
    e127 = np.zeros((128, H), np.float32)
    e127[H, :] = 1.0
    put_wb("e127", e127)
    put_wb("identb", np.eye(128, dtype=np.float32))

    fbund = np.zeros((128, FB_COLS), np.float32)

    def put_fb(nm, mat):
        c0, w, p = _FB[nm]
        assert mat.shape == (p, w), (nm, mat.shape)
        fbund[0:p, c0:c0 + w] = mat

    put_fb("GA", (2.0 * (1.0 - d_t)) * G.T)
    put_fb("GB", (-(1.0 - d_t)) * G.T)
    put_fb("o1b", (p1_b - p1_W[:, :123].sum(1)).reshape(2, 128).T.copy())
    put_fb("p2b", p2_b.reshape(H, 1))
    put_fb("dtv", np.full((H, 1), d_t, np.float32))
    put_fb("ident", np.eye(128, dtype=np.float32))

    shared = dict(
        stab=stab, kdx=_xtab(bf(kdt)), e3x=_xtab(bf(e3t)),
        wb=bf(wbund), fb=fbund.astype(np.float32), A1=bf(A1_flat),
        onesr=bf(np.ones((1, (T + 1) * BL), np.float32)),
    )

    qm = f("q_maritx")
    qmn = f("q_maritx_next")
    r = np.asarray(inputs["r"]).astype(np.float32)
    sid = ii("s_id").astype(np.int32)
    eid = ii("e_id")
    qnx = ii("q_next")

    in_maps = []
    for c in range(NCORES):
        sl = slice(c * BL, (c + 1) * BL)
        qmTc = np.ones((124, NT), np.float32)
        qmTc[0:H] = qm[sl].transpose(2, 1, 0).reshape(H, NT)
        qmnTc = qmn[sl].transpose(2, 1, 0).reshape(H, NT)
        rr2 = np.ones((2, NT), np.float32)
        rr2[0] = r[sl].T.reshape(NT)
        eidf = eid[sl].T.reshape(NT)        # t-major flat
        qnxf = qnx[sl].T.reshape(NT)
        ka, kb = _split_idx(eidf)
        ea, eb = _split_idx(qnxf)
        m = dict(shared)
        m.update(
            qmT=bf(qmTc), qmnT=bf(qmnTc), rr2=bf(rr2),
            sid=sid[sl].reshape(BL, 1),
            ixka=ka, ixkb=kb, ixea=ea, ixeb=eb,
        )
        in_maps.append(m)
    return in_maps


_NC_CACHE = {}


def kernel(**inputs):
    if "nc" not in _NC_CACHE:
        _NC_CACHE["nc"] = build_nc()
    nc = _NC_CACHE["nc"]
    in_maps = host_prep(inputs)
    res = run_bass_kernel_spmd(nc, in_maps, core_ids=list(range(NCORES)))
    out = np.concatenate([r["out"] for r in res.results], 0)   # [1024,39]
    return out.reshape(B, T, 1).astype(np.float32)


# revision 12
# speedup vs baseline: 1.7003x; 1.7003x over previous
"""Trainium2 Bass kernel for nn_DKT_89034672046889 (DKT-style recurrent net).

Strategy: data-parallel over batch across 8 NeuronCores (128 rows each).
On-device layout is feature-major ([feature, t*128+b]); host does
weight-only precompute (sigmoid tables, answer-embedding folds, gate-folded
recurrent weights) and input layout transforms (transpose/cast/shard).
Both scans are expressed with sigmoid-only activations:
  scan1 state v_t = sigma(2*a_t)        (u_t = tanh(a_t) = 2 v_t - 1)
  scan2 state p_t = (1+h_t)/2 in [0,1]  (p' = (1-z) p + z w)
so the ACT engine never switches function tables.
Table gathers use split-range int16 dma_gather(transpose=True) with
zero-row dummy slots, merged by a single add -- results land feature-major.
"""

import sys

for _p in ("/root/.axon_site/_ro/trn_rl_repo", "/opt/trn_rl_repo"):
    if _p not in sys.path:
        sys.path.append(_p)

import numpy as np
import ml_dtypes

import concourse.bacc as bacc
import concourse.mybir as mybir
import concourse.tile as tile
from concourse.bass import IndirectOffsetOnAxis
from concourse.bass_utils import run_bass_kernel_spmd

BF = mybir.dt.bfloat16
F32 = mybir.dt.float32
I32 = mybir.dt.int32
I16 = mybir.dt.int16

B, T, H, EMB = 1024, 39, 123, 256
NCORES = 8
BL = B // NCORES            # 128 batch rows per core
NT = T * BL                 # 4992 columns, t-major (n = t*128 + b)
GSZ = 512
SPLIT = 32767               # int16 index split for dma_gather

_bf16 = ml_dtypes.bfloat16

# bf16 weight-bundle column layout: name -> (col0, ncols, nparts)
_WB = {}
_c = 0
for _n, _w, _p in (("Lzz", H, 124), ("Lxp", H, 125), ("Lz", H, 125),
                   ("Lh2", H, 125), ("Az", H, H), ("Ah", H, H),
                   ("o1a", EMB, H), ("o1c", EMB, H), ("p2T", 2 * H, 128),
                   ("p3g", 1, 124), ("e127", H, 128), ("identb", 128, 128)):
    _WB[_n] = (_c, _w, _p)
    _c += _w
WB_COLS = _c
# f32 bundle
_FB = {}
_c = 0
for _n, _w, _p in (("GA", T, H), ("GB", T, H), ("o1b", 2, 128),
                   ("p2b", 1, H), ("dtv", 1, H), ("ident", 128, 128)):
    _FB[_n] = (_c, _w, _p)
    _c += _w
FB_COLS = _c


def _sigmoid(x):
    return 1.0 / (1.0 + np.exp(-x))


def _groups():
    out = []
    c = 0
    while c < NT:
        s = min(GSZ, NT - c)
        out.append((c, s))
        c += s
    return out


def build_nc(dbg=False):
    nc = bacc.Bacc(None, target_bir_lowering=False, debug=False)

    dt = nc.dram_tensor
    qmT_d = dt("qmT", [124, NT], BF, kind="ExternalInput")       # row 123 = ones
    qmnT_d = dt("qmnT", [123, NT], BF, kind="ExternalInput")
    rr2_d = dt("rr2", [2, NT], BF, kind="ExternalInput")         # [r_row; ones]
    onesr_d = dt("onesr", [1, (T + 1) * BL], BF, kind="ExternalInput")
    sid_d = dt("sid", [BL, 1], I32, kind="ExternalInput")
    eid_d = dt("eid", [BL, T], I32, kind="ExternalInput")
    qnx_d = dt("qnx", [BL, T], I32, kind="ExternalInput")
    stab_d = dt("stab", [100001, H], F32, kind="ExternalInput")
    kdtab_d = dt("kdtab", [50001, H + 1], F32, kind="ExternalInput")
    e3tab_d = dt("e3tab", [50001, EMB], F32, kind="ExternalInput")
    wb_d = dt("wb", [128, WB_COLS], BF, kind="ExternalInput")
    fb_d = dt("fb", [128, FB_COLS], F32, kind="ExternalInput")
    A1_d = dt("A1", [124, T * H], BF, kind="ExternalInput")

    out_d = dt("out", [BL, T], F32, kind="ExternalOutput")
    dbg_d = {}
    if dbg:
        for nm, shp in (("dRHS1", [125, NT]), ("dV", [124, (T + 1) * BL]),
                        ("dRHS2", [125, NT]), ("dP", [H, (T + 1) * BL]),
                        ("dO1", [128, 2 * NT]), ("dO2", [124, NT]),
                        ("dDKT", [124, NT]), ("dqmDT", [H, NT]),
                        ("dDKQ", [H, NT])):
            dbg_d[nm] = dt(nm, shp, BF, kind="ExternalOutput")
        dbg_d["dSPT"] = dt("dSPT", [H, BL], F32, kind="ExternalOutput")

    groups = _groups()

    with tile.TileContext(nc) as tc:
        with (
            tc.tile_pool(name="per", bufs=1) as per,
            tc.tile_pool(name="gat", bufs=12) as gat,
            tc.tile_pool(name="tmp", bufs=4) as tmp,
            tc.tile_pool(name="zw", bufs=3) as zwp,
            tc.tile_pool(name="psA", bufs=2, space="PSUM") as psA,
            tc.tile_pool(name="psPT", bufs=2, space="PSUM") as psPT,
            tc.tile_pool(name="psT", bufs=1, space="PSUM") as psT,
            tc.tile_pool(name="psO1", bufs=1, space="PSUM") as psO1,
            tc.tile_pool(name="psO2", bufs=1, space="PSUM") as psO2,
            tc.tile_pool(name="psO3", bufs=1, space="PSUM") as psO3,
        ):
            sync, gp, ve, se, te = nc.sync, nc.gpsimd, nc.vector, nc.scalar, nc.tensor
            SIG = mybir.ActivationFunctionType.Sigmoid
            MUL = mybir.AluOpType.mult
            ADD = mybir.AluOpType.add
            SUB = mybir.AluOpType.subtract

            # ---- persistent SBUF tiles ----
            qmT = per.tile([124, NT], BF)
            qmnT = per.tile([123, NT], BF)
            RHS1 = per.tile([125, NT], BF)
            RHS2 = per.tile([125, NT], BF)
            V = per.tile([124, (T + 1) * BL], BF)
            P = per.tile([H, (T + 1) * BL], BF)
            DKT = per.tile([124, NT], BF)
            qmDT = per.tile([H, NT], BF)
            DKQ = per.tile([H, NT], BF)
            O1 = per.tile([128, 2 * NT], BF)
            O2 = per.tile([124, NT], BF)
            sidt = per.tile([BL, 1], I32)
            eidt = per.tile([BL, T], I32)
            qnxt = per.tile([BL, T], I32)
            WB = per.tile([128, WB_COLS], BF)
            FB = per.tile([128, FB_COLS], F32)
            A1 = per.tile([124, T * H], BF)
            SPT = per.tile([H, BL], F32)
            SPD = per.tile([H, BL], BF)
            SPT4 = per.tile([H, GSZ], F32)
            OUTt = per.tile([BL, T], F32)

            def wb(nm, parts=None):
                c0, w, p = _WB[nm]
                return WB[0:(parts or p), c0:c0 + w]

            def fb(nm, parts=None):
                c0, w, p = _FB[nm]
                return FB[0:(parts or p), c0:c0 + w]

            # ---- small init ----
            ve.memset(V[0:H, 0:BL], 0.0)
            ve.memset(P[:, 0:BL], 0.5)

            # ---- loads ----
            sync.dma_start(out=sidt[:], in_=sid_d[:])
            sync.dma_start(out=eidt[:], in_=eid_d[:])
            sync.dma_start(out=qnxt[:], in_=qnx_d[:])
            sync.dma_start(out=WB[:], in_=wb_d[:])
            sync.dma_start(out=FB[:], in_=fb_d[:])
            sync.dma_start(out=A1[:], in_=A1_d[:])
            sync.dma_start(out=qmT[:], in_=qmT_d[:])
            sync.dma_start(out=qmnT[:], in_=qmnT_d[:])
            sync.dma_start(out=RHS1[123:125, :], in_=rr2_d[:])
            sync.dma_start(out=RHS2[123:125, :], in_=rr2_d[:])
            sync.dma_start(out=V[123:124, :], in_=onesr_d[:])
            sync.dma_start(out=O2[123:124, :], in_=onesr_d[:, 0:NT])

            # ---- student profile (indirect gather + PE transpose) ----
            sg = per.tile([BL, H], F32)
            gp.indirect_dma_start(
                out=sg[:], out_offset=None, in_=stab_d[:],
                in_offset=IndirectOffsetOnAxis(ap=sidt[:, 0:1], axis=0))
            pT = psT.tile([124, 128], F32, space="PSUM", tag="tp")
            te.transpose(out=pT[0:H, :], in_=sg[:], identity=fb("ident"))
            ve.tensor_copy(out=SPT[:], in_=pT[0:H, :])
            ve.tensor_scalar(out=SPD[:], in0=SPT[:], scalar1=fb("dtv"),
                             scalar2=None, op0=MUL)
            for i in range(4):
                ve.tensor_copy(out=SPT4[:, i * BL:(i + 1) * BL], in_=SPT[:])

            # ---- concept_ab + x1a -> RHS1 rows 0:123 ----
            for (c0, csz) in groups:
                pa = psA.tile([H, GSZ], F32, space="PSUM", tag="a")
                te.matmul(out=pa[:, 0:csz], lhsT=wb("Lzz"), rhs=qmT[:, c0:c0 + csz],
                          start=True, stop=True)
                ve.tensor_tensor(out=RHS1[0:H, c0:c0 + csz], in0=pa[:, 0:csz],
                                 in1=SPT4[:, 0:csz], op=MUL)

            # ---- qmDT / DKQ per group + scan1 + middle (pipelined emission) ----
            e3tiles = [None] * T
            ps1 = [None]
            for t in range(T + 4):
                if t < T:
                    kg = gat.tile([BL, H + 1], F32, tag="kg")
                    gp.indirect_dma_start(
                        out=kg[:], out_offset=None, in_=kdtab_d[:],
                        in_offset=IndirectOffsetOnAxis(ap=eidt[:, t:t + 1], axis=0))
                    eg = gat.tile([BL, EMB], F32, tag="eg")
                    gp.indirect_dma_start(
                        out=eg[:], out_offset=None, in_=e3tab_d[:],
                        in_offset=IndirectOffsetOnAxis(ap=qnxt[:, t:t + 1], axis=0))
                    e3tiles[t] = eg
                    pk = psT.tile([124, 128], F32, space="PSUM", tag="tp",
                                  name="pkt")
                    te.transpose(out=pk[:], in_=kg[:], identity=fb("ident"))
                    ve.tensor_copy(out=DKT[:, t * BL:(t + 1) * BL], in_=pk[:])
                    if t % 4 == 3 or t == T - 1:
                        g = t // 4
                        c0, csz = groups[g]
                        pd = psA.tile([H, GSZ], F32, space="PSUM", tag="a",
                                      name="pdg")
                        te.matmul(out=pd[:, 0:csz], lhsT=wb("e127", parts=124),
                                  rhs=DKT[:, c0:c0 + csz], start=True, stop=True)
                        ve.tensor_tensor(out=qmDT[:, c0:c0 + csz], in0=pd[:, 0:csz],
                                         in1=qmT[0:H, c0:c0 + csz], op=MUL)
                        ve.tensor_tensor(out=DKQ[:, c0:c0 + csz],
                                         in0=DKT[0:H, c0:c0 + csz],
                                         in1=qmT[0:H, c0:c0 + csz], op=MUL)
                    # scan1 tick t
                    gi, off = divmod(t, 4)
                    if off == 0:
                        c0 = gi * GSZ
                        csz = min(GSZ, NT - c0)
                        ps1[0] = psA.tile([H, GSZ], F32, space="PSUM", tag="a",
                                          name="ps1g")
                        te.matmul(out=ps1[0][:, 0:csz], lhsT=wb("Lxp"),
                                  rhs=RHS1[:, c0:c0 + csz], start=True, stop=True)
                    sl = slice(off * BL, (off + 1) * BL)
                    te.matmul(out=ps1[0][:, sl], lhsT=A1[:, t * H:(t + 1) * H],
                              rhs=V[:, t * BL:(t + 1) * BL], start=False, stop=True)
                    se.activation(out=V[0:H, (t + 1) * BL:(t + 2) * BL],
                                  in_=ps1[0][:, sl], func=SIG)
                # middle, lagged 4 ticks
                tm = t - 4
                if 0 <= tm < T:
                    m1 = tmp.tile([H, BL], BF, tag="m1")
                    ve.tensor_scalar(out=m1[:],
                                     in0=V[0:H, (tm + 1) * BL:(tm + 2) * BL],
                                     scalar1=fb("GA")[:, tm:tm + 1],
                                     scalar2=fb("GB")[:, tm:tm + 1],
                                     op0=MUL, op1=ADD)
                    m2 = tmp.tile([H, BL], BF, tag="m2")
                    ve.tensor_tensor(out=m2[:], in0=m1[:], in1=SPD[:], op=ADD)
                    m3 = tmp.tile([H, BL], BF, tag="m3")
                    ve.tensor_tensor(out=m3[:], in0=m2[:],
                                     in1=qmDT[:, tm * BL:(tm + 1) * BL], op=MUL)
                    ve.tensor_tensor(out=RHS2[0:H, tm * BL:(tm + 1) * BL], in0=m3[:],
                                     in1=DKQ[:, tm * BL:(tm + 1) * BL], op=SUB)

            # ---- scan2 + o1/o2 pipelined ----
            for t in range(T + 8):
                if t < T:
                    pt = psPT.tile([H, 256], F32, space="PSUM", tag="pt")
                    te.matmul(out=pt[:, 0:128], lhsT=wb("Lz"),
                              rhs=RHS2[:, t * BL:(t + 1) * BL], start=True, stop=True)
                    te.matmul(out=pt[:, 128:256], lhsT=wb("Lh2"),
                              rhs=RHS2[:, t * BL:(t + 1) * BL], start=False, stop=True)
                    te.matmul(out=pt[:, 0:128], lhsT=wb("Az"),
                              rhs=P[:, t * BL:(t + 1) * BL], start=False, stop=True)
                    te.matmul(out=pt[:, 128:256], lhsT=wb("Ah"),
                              rhs=P[:, t * BL:(t + 1) * BL], start=False, stop=True)
                    zw = zwp.tile([H, 256], BF, tag="zw")
                    se.activation(out=zw[:], in_=pt[:], func=SIG)
                    d1 = tmp.tile([H, BL], BF, tag="d1")
                    ve.tensor_tensor(out=d1[:], in0=zw[:, 128:256],
                                     in1=P[:, t * BL:(t + 1) * BL], op=SUB)
                    d2 = tmp.tile([H, BL], BF, tag="d2")
                    ve.tensor_tensor(out=d2[:], in0=zw[:, 0:128], in1=d1[:], op=MUL)
                    ve.tensor_tensor(out=P[:, (t + 1) * BL:(t + 2) * BL],
                                     in0=P[:, t * BL:(t + 1) * BL], in1=d2[:], op=ADD)
                if t >= 4 and (t - 4) % 4 == 0:
                    g = (t - 4) // 4
                    if g < len(groups):
                        c0, csz = groups[g]
                        ts_r = range(g * 4, min(g * 4 + 4, T))
                        for m in range(2):
                            po = psO1.tile([128, GSZ], F32, space="PSUM", tag="o1")
                            for i, tt in enumerate(ts_r):
                                te.matmul(out=po[:, i * BL:(i + 1) * BL],
                                          lhsT=e3tiles[tt][:, m * 128:(m + 1) * 128],
                                          rhs=fb("ident"), is_transpose=True,
                                          start=(i == 0), stop=True)
                            te.matmul(out=po[:, 0:csz],
                                      lhsT=wb("o1a")[:, m * 128:(m + 1) * 128],
                                      rhs=P[:, BL + c0:BL + c0 + csz],
                                      start=False, stop=True)
                            te.matmul(out=po[:, 0:csz],
                                      lhsT=wb("o1c")[:, m * 128:(m + 1) * 128],
                                      rhs=qmnT[:, c0:c0 + csz], start=False, stop=True)
                            se.activation(out=O1[:, m * NT + c0:m * NT + c0 + csz],
                                          in_=po[:, 0:csz], func=SIG,
                                          bias=fb("o1b")[:, m:m + 1])
                if t >= 8 and (t - 8) % 4 == 0:
                    g = (t - 8) // 4
                    if g < len(groups):
                        c0, csz = groups[g]
                        p2 = psO2.tile([H, GSZ], F32, space="PSUM", tag="o2")
                        te.matmul(out=p2[:, 0:csz], lhsT=wb("p2T")[:, 0:H],
                                  rhs=O1[:, c0:c0 + csz], start=True, stop=True)
                        te.matmul(out=p2[:, 0:csz], lhsT=wb("p2T")[:, H:2 * H],
                                  rhs=O1[:, NT + c0:NT + c0 + csz],
                                  start=False, stop=True)
                        se.activation(out=O2[0:H, c0:c0 + csz], in_=p2[:, 0:csz],
                                      func=SIG, bias=fb("p2b"))

            # ---- o3 ----
            p3 = psO3.tile([128, T], F32, space="PSUM", tag="o3")
            for t in range(T):
                te.matmul(out=p3[:, t:t + 1], lhsT=O2[:, t * BL:(t + 1) * BL],
                          rhs=wb("p3g"), start=(t == 0), stop=True)
            se.activation(out=OUTt[:], in_=p3[:], func=SIG)
            sync.dma_start(out=out_d[:], in_=OUTt[:])
            if dbg:
                for nm, src_tile in (("dRHS1", RHS1), ("dV", V), ("dRHS2", RHS2),
                                     ("dP", P), ("dO1", O1), ("dO2", O2),
                                     ("dDKT", DKT), ("dqmDT", qmDT),
                                     ("dDKQ", DKQ)):
                    sync.dma_start(out=dbg_d[nm][:], in_=src_tile[:])
                sync.dma_start(out=dbg_d["dSPT"][:], in_=SPT[:])

    nc.finalize()
    return nc


def _wrap_idx(a):
    """flat [NT] int16 -> [128, NT//16] wrapped+replicated layout."""
    return np.ascontiguousarray(np.tile(a.reshape(NT // 16, 16).T, (8, 1)))


def _split_idx(idx):
    """flat indices -> (idxA, idxB) int16 with zero-row dummies."""
    a = np.where(idx < SPLIT, idx + 1, 0).astype(np.int16)
    b = np.where(idx >= SPLIT, idx - SPLIT + 1, 0).astype(np.int16)
    return _wrap_idx(a), _wrap_idx(b)


def _xtab(tbl):
    """[R, W] table -> [R+2, W] with zero rows at 0 and SPLIT+1."""
    R, W = tbl.shape
    out = np.zeros((R + 2, W), tbl.dtype)
    out[1:SPLIT + 1] = tbl[0:SPLIT]
    out[SPLIT + 2:] = tbl[SPLIT:]
    return out


def host_prep(inputs):
    """Weight-only precompute + input layout transforms. Returns in_maps."""
    f = lambda k: np.asarray(inputs[k], np.float32)
    ii = lambda k: np.asarray(inputs[k]).astype(np.int64)

    d_t = float(f("d_t")[0])
    d_e = float(f("d_e")[0])
    W_ih, b_ih = f("W_ih"), f("b_ih")
    W_hh, b_hh = f("W_hh"), f("b_hh")
    W_z, b_z = f("W_z"), f("b_z")
    W_h, b_h = f("W_h"), f("b_h")
    answer_W = f("answer_W")
    zz_W, zz_b = f("zz_W"), f("zz_b")
    p1_W, p1_b = f("p1_W"), f("p1_b")
    p2_W, p2_b = f("p2_W"), f("p2_b")
    p3_W, p3_b = f("p3_W"), f("p3_b")
    W_tg, b_tg = f("W_tg"), f("b_tg")

    tvec = np.arange(T, dtype=np.float32)[:, None]
    G = _sigmoid(tvec * W_tg[:, 0][None, :] + b_tg)          # [T,123]

    stab = _sigmoid(f("student_W")).astype(np.float32)
    D_tab = _sigmoid(f("e_disc_W")[:, 0]) * d_e
    kdtab = np.concatenate(
        [_sigmoid(f("k_diff_W")) * D_tab[:, None], D_tab[:, None]], 1
    ).astype(np.float32)
    e3tab = (f("emb_problem") @ p1_W[:, 123:379].T).astype(np.float32)

    def fold(Wm, bias):
        ap = answer_W @ Wm[:, 123:379].T
        return ap[0] + bias, ap[1] - ap[0]
    c0_ih, dl_ih = fold(W_ih, b_ih)
    c0_z, dl_z = fold(W_z, b_z)
    c0_h, dl_h = fold(W_h, b_h)
    Wz_h = W_z[:, 379:502]
    Wh_h = W_h[:, 379:502]

    A1_flat = np.zeros((124, T * H), np.float32)
    A1_flat[123, 0:H] = 2 * b_hh
    for t in range(1, T):
        g = G[t - 1]
        A1_flat[:123, t * H:(t + 1) * H] = 4.0 * g[:, None] * W_hh.T
        A1_flat[123, t * H:(t + 1) * H] = 2 * b_hh - 2.0 * (W_hh @ g)

    bf = lambda x: np.ascontiguousarray(x, np.float32).astype(_bf16)

    wbund = np.zeros((128, WB_COLS), np.float32)

    def put_wb(nm, mat):
        c0, w, p = _WB[nm]
        assert mat.shape == (p, w), (nm, mat.shape)
        wbund[0:p, c0:c0 + w] = mat

    put_wb("Lzz", np.concatenate([zz_W.T, zz_b[None]], 0))
    put_wb("Lxp", np.concatenate(
        [2 * W_ih[:, :123].T, 2 * dl_ih[None], 2 * c0_ih[None]], 0))
    put_wb("Lz", np.concatenate(
        [W_z[:, :123].T, dl_z[None], (c0_z - Wz_h.sum(1))[None]], 0))
    put_wb("Lh2", np.concatenate(
        [2 * W_h[:, :123].T, 2 * dl_h[None], (2 * c0_h - 2 * Wh_h.sum(1))[None]], 0))
    put_wb("Az", 2.0 * Wz_h.T)
    put_wb("Ah", 4.0 * Wh_h.T)
    put_wb("o1a", 2.0 * p1_W[:, :123].T)
    put_wb("o1c", p1_W[:, 379:502].T)
    p2Tm = np.zeros((128, 2 * H), np.float32)
    p2Tm[:, 0:H] = p2_W.T[0:128]
    p2Tm[:, H:2 * H] = p2_W.T[128:256]
    put_wb("p2T", p2Tm)
    put_wb("p3g", np.concatenate([p3_W[0], p3_b]).reshape(124, 1))
    e127 = np.zeros((128, H), np.float32)
    e127[H, :] = 1.0
    put_wb("e127", e127)
    put_wb("identb", np.eye(128, dtype=np.float32))

    fbund = np.zeros((128, FB_COLS), np.float32)

    def put_fb(nm, mat):
        c0, w, p = _FB[nm]
        assert mat.shape == (p, w), (nm, mat.shape)
        fbund[0:p, c0:c0 + w] = mat

    put_fb("GA", (2.0 * (1.0 - d_t)) * G.T)
    put_fb("GB", (-(1.0 - d_t)) * G.T)
    put_fb("o1b", (p1_b - p1_W[:, :123].sum(1)).reshape(2, 128).T.copy())
    put_fb("p2b", p2_b.reshape(H, 1))
    put_fb("dtv", np.full((H, 1), d_t, np.float32))
    put_fb("ident", np.eye(128, dtype=np.float32))

    shared = dict(
        stab=stab, kdtab=kdtab, e3tab=e3tab,
        wb=bf(wbund), fb=fbund.astype(np.float32), A1=bf(A1_flat),
        onesr=bf(np.ones((1, (T + 1) * BL), np.float32)),
    )

    qm = f("q_maritx")
    qmn = f("q_maritx_next")
    r = np.asarray(inputs["r"]).astype(np.float32)
    sid = ii("s_id").astype(np.int32)
    eid = ii("e_id")
    qnx = ii("q_next")

    in_maps = []
    for c in range(NCORES):
        sl = slice(c * BL, (c + 1) * BL)
        qmTc = np.ones((124, NT), np.float32)
        qmTc[0:H] = qm[sl].transpose(2, 1, 0).reshape(H, NT)
        qmnTc = qmn[sl].transpose(2, 1, 0).reshape(H, NT)
        rr2 = np.ones((2, NT), np.float32)
        rr2[0] = r[sl].T.reshape(NT)
        m = dict(shared)
        m.update(
            qmT=bf(qmTc), qmnT=bf(qmnTc), rr2=bf(rr2),
            sid=sid[sl].reshape(BL, 1),
            eid=eid[sl].astype(np.int32), qnx=qnx[sl].astype(np.int32),
        )
        in_maps.append(m)
    return in_maps


_NC_CACHE = {}


def kernel(**inputs):
    if "nc" not in _NC_CACHE:
        _NC_CACHE["nc"] = build_nc()
    nc = _NC_CACHE["nc"]
    in_maps = host_prep(inputs)
    res = run_bass_kernel_spmd(nc, in_maps, core_ids=list(range(NCORES)))
    out = np.concatenate([r["out"] for r in res.results], 0)   # [1024,39]
    return out.reshape(B, T, 1).astype(np.float32)
